# revision 1
# baseline (speedup 1.0000x reference)
"""TRN2 Bass kernel for block-sparse attention (nn_BlockSparseAttention).

kernel(**inputs) takes the FULL unsharded inputs (x [4,4096,1024], Wq/Wk/Wv/Wo
[1024,1024], bq/bk/bv/bo [1024]) and returns the full output [4,4096,1024].

Sharding: 8 cores = 4 batches x 2 head-halves (8 heads each). Each core
computes QKV projections, block-sparse attention, and a partial
out-projection [4096,1024]; the host sums the two half-partials plus bo.

v2 design (group-major, fully SBUF-resident attention):
  - projections in fp16 (full PE rate, double-buffered LDWEIGHTS); q^T/k^T
    land in SBUF [d, tok] bf16, v lands in a 64-token-SHIFTED chunk layout
    vs [128, 33 chunks x 130/j] so that every 128-aligned query group's
    3-block window is exactly two aligned key chunks
  - group g (tokens [128g, 128g+128)): S^T = matmul(kt, qt) per piece/head
    into one PSUM bank [128,512]; ONE exp op per (g,j); PV splits per
    query-half so masked corners are never read (no memsets); v carries a
    ones column per head so PV emits the softmax denominator as column 64
  - division is per-partition (reciprocal + tensor_scalar_mul) in [q, d]
    layout, then a PE transpose yields A^T bf16 for the out-projection
  - block 0 attends globally: separate 33-piece S/PV accumulation per j,
    interleaved between groups to fill Act-engine slack
  - the out-projection (bf16) for chunk g-2 is interleaved into group g's
    PE stream: the PE never idles, so the DVFS p-state stays at 2.4 GHz
"""
import os

import numpy as np

import concourse.bass as bass
import concourse.tile as tile
from concourse import mybir

F32 = mybir.dt.float32
F16 = mybir.dt.float16
BF16 = mybir.dt.bfloat16
AF = mybir.ActivationFunctionType
SCALE = 1.0 / 8.0  # 1/sqrt(Dh=64)

N_CORES = 8
LAST_EXEC_NS = None


def _split_sync_waits(nc, cap=1):
    """This walrus build rejects >cap sync waits on one instruction; move
    excess waits onto same-engine no-ops placed just before (waits only
    become stricter in order, so this is semantics-preserving)."""
    for fn in nc.m.functions:
        for bb in fn.blocks:
            out = []
            for inst in bb.instructions:
                si = inst.sync_info
                waits = list(si.on_wait) if si and si.on_wait else []
                if len(waits) > cap:
                    extra, keep = waits[:-cap], waits[-cap:]
                    for i in range(0, len(extra), cap):
                        nop = mybir.InstNoOp(
                            name=nc.get_next_instruction_name(),
                            engine=inst.engine,
                            ins=[],
                            outs=[],
                            sync_info=mybir.SyncInfo(
                                on_wait=extra[i : i + cap], on_update=[]
                            ),
                        )
                        nc.register_instruction(nop)
                        out.append(nop)
                    si.on_wait = keep
                out.append(inst)
            bb.instructions[:] = out


def build_kernel(NT=4096, DM=1024, HL=8, DMO=1024):
    """One-core program; SPMD across 8 cores with different input slices."""
    STAGE = int(os.environ.get("BSATTN_STAGE", "5"))
    DO = HL * 64          # local head dims (512)
    DOV = HL * 65         # v with interleaved ones columns (520)
    KC = DM // 128        # d_model chunks (8)
    NJ = DO // 128        # head pairs (4)
    STOK = 512
    NS = NT // STOK       # 8
    NG = NT // 128        # 32 token chunks / query groups
    NCH = NG + 1          # 33 shifted v chunks

    nc = bass.Bass()
    # cache-buster: some compile caches in this stack key on the HLO
    # interface only (not the embedded BIR), so a stale executable from a
    # previous kernel version can be wrongly reused. A source-hash-sized
    # dummy input makes every kernel edit change the HLO signature.
    import hashlib
    _nonce = 1 + int(hashlib.sha256(open(__file__, "rb").read()).hexdigest(), 16) % 509
    nonce_d = nc.dram_tensor("nonce", [1, _nonce], F32, kind="ExternalInput")
    xt_d = nc.dram_tensor("xt", [DM, NT], F16, kind="ExternalInput")
    wq_d = nc.dram_tensor("wq", [DM, DO], F16, kind="ExternalInput")
    wk_d = nc.dram_tensor("wk", [DM, DO], F16, kind="ExternalInput")
    wv_d = nc.dram_tensor("wvp", [DM, DOV], F16, kind="ExternalInput")
    wo_d = nc.dram_tensor("wo", [DO, DMO], BF16, kind="ExternalInput")
    bq_d = nc.dram_tensor("bq", [128, NJ], F32, kind="ExternalInput")
    bk_d = nc.dram_tensor("bk", [128, NJ], F32, kind="ExternalInput")
    bvb_d = nc.dram_tensor("bvb", [128, DOV], F32, kind="ExternalInput")
    idn_d = nc.dram_tensor("idn", [128, 128], BF16, kind="ExternalInput")
    lm_d = nc.dram_tensor("lm", [128, 128], BF16, kind="ExternalInput")
    rm_d = nc.dram_tensor("rm", [128, 512], BF16, kind="ExternalInput")
    y_d = nc.dram_tensor("y", [NT, DMO], F32, kind="ExternalOutput")

    with tile.TileContext(nc) as tc, nc.allow_low_precision(
        reason="attention operands intentionally bf16/fp16; matmul accum f32"
    ):
        from contextlib import ExitStack

        with ExitStack() as ctx:
            pers = ctx.enter_context(tc.tile_pool(name="pers", bufs=1))
            kts = [pers.tile([128, NT], BF16, tag=f"kt{j}", name=f"kt{j}")
                   for j in range(NJ)]
            qts = [pers.tile([128, NT], BF16, tag=f"qt{j}", name=f"qt{j}")
                   for j in range(NJ)]
            # shifted v: chunk m = tokens [128m-64, 128m+64); per-j cols 130
            vs = pers.tile([128, NCH * DOV], BF16, tag="vs")
            idn = pers.tile([128, 128], BF16, tag="idn")
            lm = pers.tile([128, 128], BF16, tag="lm")
            warm = pers.tile([128, 512], F16, tag="warm")
            rm = pers.tile([128, 512], BF16, tag="rm")
            # block-0 PV accumulators (SBUF, accumulated batch-wise)
            oq0 = [pers.tile([64, 130], F32, tag=f"oq{j}", name=f"oq{j}")
                   for j in range(NJ)]
            nc.sync.dma_start(idn[:], idn_d[:])
            nc.sync.dma_start(lm[:], lm_d[:])
            nc.sync.dma_start(rm[:], rm_d[:])
            # zero the phantom halves of the shifted-v layout so padded
            # full-row PV chains multiply by 0 instead of garbage
            nc.vector.memset(vs[0:64, 0:DOV], 0.0)
            nc.vector.memset(vs[64:128, NG * DOV : (NG + 1) * DOV], 0.0)

            # ---------------- phase 1: projections ----------------
            with (
                tc.tile_pool(name="p1w", bufs=1) as p1w,
                tc.tile_pool(name="p1x", bufs=2) as p1x,
                tc.tile_pool(name="p1v", bufs=3) as p1v,
                tc.tile_pool(name="p1ps", bufs=3, space="PSUM") as p1ps,
                tc.tile_pool(name="p1psv", bufs=4, space="PSUM") as p1psv,
            ):
                wqs = p1w.tile([128, KC * DO], F16, tag="wqs")
                wks = p1w.tile([128, KC * DO], F16, tag="wks")
                wvs = p1w.tile([128, KC * DOV], F16, tag="wvs")
                bqs = p1w.tile([128, NJ], F32, tag="bqs")
                bks = p1w.tile([128, NJ], F32, tag="bks")
                bvbs = p1w.tile([128, DOV], F32, tag="bvbs")
                xts0 = p1x.tile([128, KC * STOK], F16, tag="xts", name="xts0")
                for c in range(KC):
                    nc.sync.dma_start(
                        xts0[:, c * STOK : (c + 1) * STOK],
                        xt_d[c * 128 : (c + 1) * 128, 0:STOK],
                    )
                for c in range(KC):
                    r = slice(c * 128, (c + 1) * 128)
                    nc.sync.dma_start(wqs[:, c * DO : (c + 1) * DO], wq_d[r, :])
                nc.sync.dma_start(bqs[:], bq_d[:])
                for c in range(KC):
                    r = slice(c * 128, (c + 1) * 128)
                    nc.sync.dma_start(wks[:, c * DO : (c + 1) * DO], wk_d[r, :])
                nc.sync.dma_start(bks[:], bk_d[:])
                for c in range(KC):
                    r = slice(c * 128, (c + 1) * 128)
                    nc.sync.dma_start(wvs[:, c * DOV : (c + 1) * DOV], wv_d[r, :])
                nc.sync.dma_start(bvbs[:], bvb_d[:])

                nc.sync.dma_start(warm[:], wq_d[0:128, 0:512])
                for s in range(NS):
                    ts = slice(s * STOK, (s + 1) * STOK)
                    if s == 0:
                        xts = xts0
                    else:
                        xts = p1x.tile([128, KC * STOK], F16, tag="xts")
                        for c in range(KC):
                            nc.sync.dma_start(
                                xts[:, c * STOK : (c + 1) * STOK],
                                xt_d[c * 128 : (c + 1) * 128, ts],
                            )
                    for (wsb, bsb, dsts) in ((wqs, bqs, qts), (wks, bks, kts)):
                        for j in range(NJ):
                            ps = p1ps.tile([128, STOK], F32, tag="ps", name="ps")
                            for c in range(KC):
                                nc.tensor.matmul(
                                    ps[:],
                                    wsb[:, c * DO + j * 128 : c * DO + (j + 1) * 128],
                                    xts[:, c * STOK : (c + 1) * STOK],
                                    start=(c == 0),
                                    stop=(c == KC - 1),
                                )
                            nc.scalar.activation(
                                dsts[j][:, ts], ps[:], AF.Identity,
                                bias=bsb[:, j : j + 1],
                            )
                    for t in range(STOK // 128):
                        T = 4 * s + t
                        val = p1v.tile([128, DOV], BF16, tag="val")
                        for (o, wd) in ((0, 260), (260, 260)):
                            psv = p1psv.tile([128, 260], F32, tag="psv")
                            for c in range(KC):
                                nc.tensor.matmul(
                                    psv[:, 0:wd],
                                    xts[:, c * STOK + t * 128 : c * STOK + (t + 1) * 128],
                                    wvs[:, c * DOV + o : c * DOV + o + wd],
                                    start=(c == 0),
                                    stop=(c == KC - 1),
                                )
                            nc.vector.tensor_add(
                                val[:, o : o + wd], bvbs[:, o : o + wd], psv[:, 0:wd]
                            )
                        # scatter into shifted-chunk layout:
                        # rows 0:64 (tokens 128T..+64) -> chunk T upper half
                        # rows 64:128 -> chunk T+1 lower half
                        nc.sync.dma_start(
                            vs[64:128, T * DOV : (T + 1) * DOV], val[0:64, :]
                        )
                        nc.sync.dma_start(
                            vs[0:64, (T + 1) * DOV : (T + 2) * DOV], val[64:128, :]
                        )

            # ---------------- phase 2: attention + out-projection ----------
            with (
                tc.tile_pool(name="p2e", bufs=12) as p2e,
                tc.tile_pool(name="p2pv", bufs=4) as p2pv,
                tc.tile_pool(name="p2as", bufs=12) as p2as,
                tc.tile_pool(name="p2di", bufs=10) as p2di,
                tc.tile_pool(name="p2at", bufs=3) as p2at,
                tc.tile_pool(name="p2y", bufs=3) as p2y,
                tc.tile_pool(name="p2w", bufs=1) as p2w,
                tc.tile_pool(name="psS", bufs=3, space="PSUM") as psS_p,
                tc.tile_pool(name="psPV", bufs=2, space="PSUM") as psPV_p,
                tc.tile_pool(name="psY", bufs=2, space="PSUM") as psY_p,
                tc.tile_pool(name="psT", bufs=1, space="PSUM") as psT_p,
            ):
                wos = [p2w.tile([128, DMO], BF16, tag=f"wo{j}", name=f"wo{j}")
                       for j in range(NJ)]
                for j in range(NJ):
                    nc.sync.dma_start(wos[j][:], wo_d[j * 128 : (j + 1) * 128, :])

                at_tiles = {}   # g -> [tile per j]
                as_tiles = {}   # g -> [astage per j]

                def vcol(m, j, hh):
                    return m * DOV + j * 130 + hh * 65

                def emit_S(g):
                    """S^T + corner masking for group g: one PSUM bank per
                    (j-pair, head-half), layout [jA | jB | j'A | j'B] x 128.
                    Masking is a 5th matmul in the accumulation chain adding
                    -1000 to the two invalid 64x64 corners per j (rank-2
                    outer product via constant lm/rm), so exp gives 0 there
                    and no per-group memsets are needed. The edge group
                    (knb<128) falls back to split exp + gpsimd memsets.
                    Returns ets[(jpair, hh)]."""
                    ka = 128 * g - 64
                    kb = 128 * g + 64
                    knb = min(128, NT - kb)
                    qs = slice(128 * g, 128 * g + 128)
                    ets = {}
                    for jp in range(NJ // 2):
                        for hh in (0, 1):
                            hr = slice(hh * 64, hh * 64 + 64)
                            ps = psS_p.tile([128, 512], F32, tag="psS",
                                            name="psS")
                            chain = knb == 128
                            for jj in range(2):
                                j = 2 * jp + jj
                                nc.tensor.matmul(
                                    ps[:, jj * 256 : jj * 256 + 128],
                                    kts[j][hr, ka : ka + 128], qts[j][hr, qs],
                                    start=(not chain) or (jj == 0), stop=not chain,
                                )
                                nc.tensor.matmul(
                                    ps[0:knb, jj * 256 + 128 : jj * 256 + 256],
                                    kts[j][hr, kb : kb + knb], qts[j][hr, qs],
                                    start=not chain, stop=not chain,
                                )
                            et = p2e.tile([128, 512], BF16, tag="et", name="et")
                            if chain:
                                nc.tensor.matmul(
                                    ps[:, 64:448], lm[hr, :], rm[hr, 64:448],
                                    start=False, stop=True,
                                )
                                nc.scalar.activation(et[:], ps[:], AF.Exp,
                                                     scale=SCALE)
                            else:
                                for jj in range(2):
                                    nc.scalar.activation(
                                        et[:, jj * 256 : jj * 256 + 128],
                                        ps[:, jj * 256 : jj * 256 + 128],
                                        AF.Exp, scale=SCALE,
                                    )
                                    nc.scalar.activation(
                                        et[0:knb, jj * 256 + 128 : jj * 256 + 256],
                                        ps[0:knb, jj * 256 + 128 : jj * 256 + 256],
                                        AF.Exp, scale=SCALE,
                                    )
                                    nc.gpsimd.memset(
                                        et[knb:128, jj * 256 + 128 : jj * 256 + 256],
                                        0.0,
                                    )
                                    nc.gpsimd.memset(
                                        et[0:64, jj * 256 + 64 : jj * 256 + 128],
                                        0.0,
                                    )
                            for jj in range(2):
                                ets[(2 * jp + jj, hh)] = (et, jj * 256)
                    return ets

                def emit_S2(g1):
                    """S^T for the group pair (g1, g1+1): adjacent groups
                    share their middle key chunk, so one 256-col matmul
                    covers both. One PSUM bank per (j, head-half):
                    [ca x q1 | cm x q12 (256) | cb x q2]; the mask matmul
                    corner layout is identical to the single-group case.
                    Returns (ets_g1, ets_g2) keyed (j, hh) -> (tile, off)."""
                    g2 = g1 + 1
                    ka = 128 * g1 - 64
                    km = 128 * g1 + 64
                    kb = 128 * g2 + 64
                    e1, e2 = {}, {}
                    for j in range(NJ):
                        for hh in (0, 1):
                            hr = slice(hh * 64, hh * 64 + 64)
                            ps = psS_p.tile([128, 512], F32, tag="psS",
                                            name="psS2")
                            nc.tensor.matmul(
                                ps[:, 0:128],
                                kts[j][hr, ka : ka + 128],
                                qts[j][hr, 128 * g1 : 128 * g1 + 128],
                                start=True, stop=False,
                            )
                            nc.tensor.matmul(
                                ps[:, 128:384],
                                kts[j][hr, km : km + 128],
                                qts[j][hr, 128 * g1 : 128 * g1 + 256],
                                start=False, stop=False,
                            )
                            nc.tensor.matmul(
                                ps[:, 384:512],
                                kts[j][hr, kb : kb + 128],
                                qts[j][hr, 128 * g2 : 128 * g2 + 128],
                                start=False, stop=False,
                            )
                            nc.tensor.matmul(
                                ps[:, 64:448], lm[hr, :], rm[hr, 64:448],
                                start=False, stop=True,
                            )
                            et = p2e.tile([128, 512], BF16, tag="et", name="et2")
                            nc.scalar.activation(et[:], ps[:], AF.Exp,
                                                 scale=SCALE)
                            e1[(j, hh)] = (et, 0)
                            e2[(j, hh)] = (et, 256)
                    return e1, e2

                def emit_PV(g, ets):
                    """merged PV, batched reciprocal, division on DVE."""
                    as_tiles[g] = []
                    for j in range(NJ):
                        jj = j % 2
                        if jj == 0:
                            pv = psPV_p.tile([128, 512], F32, tag="psPV",
                                             name="psPV")
                        c0 = jj * 130
                        for hh in (0, 1):
                            et, off = ets[(j, hh)]
                            co = c0 + hh * 65
                            nc.tensor.matmul(
                                pv[:, co : co + 65],
                                et[:, off : off + 128],
                                vs[:, vcol(g, j, hh) : vcol(g, j, hh) + 65],
                                start=True, stop=False,
                            )
                            nc.tensor.matmul(
                                pv[:, co : co + 65],
                                et[:, off + 128 : off + 256],
                                vs[:, vcol(g + 1, j, hh) : vcol(g + 1, j, hh) + 65],
                                start=False, stop=True,
                            )
                        if jj == 1:
                            dinv = p2di.tile([128, 4], F32, tag="dinv",
                                             name="dinv")
                            nc.vector.reciprocal(
                                dinv[:],
                                pv[:, 0:260].rearrange(
                                    "p (h c) -> p h c", c=65
                                )[:, :, 64:65],
                            )
                            for j2 in (j - 1, j):
                                ast = p2as.tile([128, 128], BF16, tag="ast",
                                                name="ast")
                                cb = (j2 % 2) * 130
                                for hh in (0, 1):
                                    nc.vector.tensor_scalar_mul(
                                        ast[:, hh * 64 : hh * 64 + 64],
                                        pv[:, cb + hh * 65 : cb + hh * 65 + 64],
                                        dinv[:, (j2 % 2) * 2 + hh : (j2 % 2) * 2 + hh + 1],
                                    )
                                as_tiles[g].append(ast)

                def emit_T(g):
                    """PE-transpose astage -> one A^T tile [128, 4*128]."""
                    pt = psT_p.tile([128, 512], BF16, tag="psT", name="psT")
                    for j in range(NJ):
                        nc.tensor.transpose(
                            pt[:, j * 128 : (j + 1) * 128], as_tiles[g][j], idn
                        )
                    att = p2at.tile([128, 512], BF16, tag="at", name="at")
                    nc.vector.tensor_copy(att[:], pt[:])
                    at_tiles[g] = att
                    del as_tiles[g]

                def emit_outproj(g):
                    for n in range(DMO // 512):
                        py = psY_p.tile([128, 512], F32, tag="psY", name="psY")
                        for j in range(NJ):
                            nc.tensor.matmul(
                                py[:],
                                at_tiles[g][:, j * 128 : (j + 1) * 128],
                                wos[j][:, n * 512 : n * 512 + 512],
                                start=(j == 0), stop=(j == NJ - 1),
                            )
                        ysb = p2y.tile([128, 512], F32, tag="ysb", name="ysb")
                        if (g + n) % 2 == 0:
                            nc.scalar.copy(ysb[:], py[:])
                        else:
                            nc.vector.tensor_copy(ysb[:], py[:])
                        nc.sync.dma_start(
                            y_d[g * 128 : (g + 1) * 128, n * 512 : n * 512 + 512],
                            ysb[:],
                        )
                    del at_tiles[g]

                # ----- block-0 global pass, batched, interleaved below -----
                def q0_batch(j, b):
                    """b=0: edge piece 0; b=1..4: middle pieces 8b-7..8b;
                    b=5: edge piece 32. Per-config PSUM banks; PV chains are
                    closed per batch and accumulated into SBUF oq0[j]."""
                    if b == 0:
                        pieces = [0]
                    elif b == 5:
                        pieces = [NG]
                    else:
                        pieces = list(range(8 * b - 7, min(8 * b, NG - 1) + 1))
                    eqs = {}
                    for hh in (0, 1):
                        hr = slice(hh * 64, hh * 64 + 64)
                        ps = psS_p.tile([128, 512], F32, tag="psS", name="psq")
                        eq = p2e.tile([128, 512], BF16, tag="et", name="eq")
                        if b == 0:
                            # keys 0:64 live on partitions 64:128 (chunk 0)
                            nc.tensor.matmul(
                                ps[64:128, 0:64], kts[j][hr, 0:64],
                                qts[j][hr, 0:64], start=True, stop=True,
                            )
                            nc.scalar.activation(
                                eq[64:128, 0:64], ps[64:128, 0:64],
                                AF.Exp, scale=SCALE,
                            )
                            nc.gpsimd.memset(eq[0:64, 0:64], 0.0)
                        elif b == 5:
                            # keys 4032:4096 on partitions 0:64 (chunk 32)
                            nc.tensor.matmul(
                                ps[0:64, 0:64], kts[j][hr, NT - 64 : NT],
                                qts[j][hr, 0:64], start=True, stop=True,
                            )
                            nc.scalar.activation(
                                eq[0:64, 0:64], ps[0:64, 0:64],
                                AF.Exp, scale=SCALE,
                            )
                            nc.gpsimd.memset(eq[64:128, 0:64], 0.0)
                        else:
                            for c, m in enumerate(pieces):
                                nc.tensor.matmul(
                                    ps[:, c * 64 : c * 64 + 64],
                                    kts[j][hr, 128 * m - 64 : 128 * m + 64],
                                    qts[j][hr, 0:64], start=True, stop=True,
                                )
                            nc.scalar.activation(
                                eq[:, 0 : len(pieces) * 64],
                                ps[:, 0 : len(pieces) * 64],
                                AF.Exp, scale=SCALE,
                            )
                        eqs[hh] = eq
                    pv = psPV_p.tile([128, 512], F32, tag="psPV", name="psq0")
                    for hh in (0, 1):
                        eq = eqs[hh]
                        for c, m in enumerate(pieces):
                            nc.tensor.matmul(
                                pv[0:64, hh * 65 : hh * 65 + 65],
                                eq[:, c * 64 : c * 64 + 64],
                                vs[:, vcol(m, j, hh) : vcol(m, j, hh) + 65],
                                start=(c == 0), stop=(c == len(pieces) - 1),
                            )
                    if b == 0:
                        nc.vector.tensor_copy(oq0[j][:], pv[0:64, 0:130])
                    else:
                        nc.vector.tensor_add(oq0[j][:], oq0[j][:], pv[0:64, 0:130])

                NB_Q0 = 6
                q0_tasks = [(j, b) for j in range(NJ) for b in range(NB_Q0)]
                g0_pend = [True]
                q0_done = 0

                # ----- group 0 (block 1 local) + block-0 division -----
                def emit_g0():
                    as_tiles[0] = []
                    # per-(piece, head-half) S banks so each keeps one config
                    ega, egb = {}, {}
                    for hh in (0, 1):
                        hr = slice(hh * 64, hh * 64 + 64)
                        psA = psS_p.tile([128, 512], F32, tag="psS", name="g0a")
                        psB = psS_p.tile([128, 512], F32, tag="psS", name="g0b")
                        for j in range(NJ):
                            nc.tensor.matmul(
                                psA[64:128, j * 64 : j * 64 + 64],
                                kts[j][hr, 0:64], qts[j][hr, 64:128],
                                start=True, stop=True,
                            )
                            nc.tensor.matmul(
                                psB[:, j * 64 : j * 64 + 64],
                                kts[j][hr, 64:192], qts[j][hr, 64:128],
                                start=True, stop=True,
                            )
                        ea = p2e.tile([128, 256], BF16, tag="et", name="ea")
                        eb = p2e.tile([128, 256], BF16, tag="et", name="eb")
                        nc.scalar.activation(
                            ea[64:128, :], psA[64:128, 0:256], AF.Exp, scale=SCALE
                        )
                        nc.gpsimd.memset(ea[0:64, :], 0.0)
                        nc.scalar.activation(eb[:], psB[:, 0:256], AF.Exp,
                                             scale=SCALE)
                        ega[hh], egb[hh] = ea, eb
                    for j in range(NJ):
                        jj = j % 2
                        if jj == 0:
                            pv0 = psPV_p.tile([128, 512], F32, tag="psPV",
                                              name="pv0")
                        c0 = jj * 130
                        for hh in (0, 1):
                            nc.tensor.matmul(
                                pv0[64:128, c0 + hh * 65 : c0 + hh * 65 + 65],
                                ega[hh][:, j * 64 : j * 64 + 64],
                                vs[:, vcol(0, j, hh) : vcol(0, j, hh) + 65],
                                start=True, stop=False,
                            )
                            nc.tensor.matmul(
                                pv0[64:128, c0 + hh * 65 : c0 + hh * 65 + 65],
                                egb[hh][:, j * 64 : j * 64 + 64],
                                vs[:, vcol(1, j, hh) : vcol(1, j, hh) + 65],
                                start=False, stop=True,
                            )
                        dinv = p2di.tile([128, 2], F32, tag="dinv", name="dinv0")
                        for hh in (0, 1):
                            nc.vector.reciprocal(
                                dinv[64:128, hh : hh + 1],
                                pv0[64:128, c0 + hh * 65 + 64 : c0 + hh * 65 + 65],
                            )
                        ast = p2as.tile([128, 128], BF16, tag="ast", name="ast0")
                        for hh in (0, 1):
                            nc.vector.tensor_scalar_mul(
                                ast[64:128, hh * 64 : hh * 64 + 64],
                                pv0[64:128, c0 + hh * 65 : c0 + hh * 65 + 64],
                                dinv[64:128, hh : hh + 1],
                            )
                        # block-0 rows 0:64 from the accumulated global pass
                        dq = p2di.tile([128, 2], F32, tag="dinv", name="dq")
                        for hh in (0, 1):
                            nc.vector.reciprocal(
                                dq[0:64, hh : hh + 1],
                                oq0[j][:, hh * 65 + 64 : hh * 65 + 65],
                            )
                        for hh in (0, 1):
                            nc.vector.tensor_scalar_mul(
                                ast[0:64, hh * 64 : hh * 64 + 64],
                                oq0[j][:, hh * 65 : hh * 65 + 64],
                                dq[0:64, hh : hh + 1],
                            )
                        as_tiles[0].append(ast)
                # ----- main pipeline over groups 1..31 -----
                # stages trail by one group each so the Act-engine exp
                # latency of group g hides behind a full group of PE work
                ets_store = {}
                for g in range(1, NG if STAGE >= 2 else 0):
                    if g == NG - 1:
                        ets_store[g] = emit_S(g)
                    elif g % 2 == 1:
                        ets_store[g], ets_store[g + 1] = emit_S2(g)
                    if g >= 2:
                        emit_PV(g - 1, ets_store.pop(g - 1))
                    if g >= 4:
                        emit_T(g - 3)
                    if g >= 5:
                        emit_outproj(g - 4)
                    # interleave block-0 batches into engine slack
                    if STAGE >= 3:
                        want = min(len(q0_tasks), (len(q0_tasks) * g) // 20)
                        while q0_done < want:
                            q0_batch(*q0_tasks[q0_done])
                            q0_done += 1
                        if STAGE >= 4 and q0_done == len(q0_tasks) and g0_pend[0]:
                            g0_pend[0] = False
                            emit_g0()
                            emit_T(0)
                            emit_outproj(0)
                if STAGE >= 2:
                    emit_PV(NG - 1, ets_store.pop(NG - 1))
                    for gg in (NG - 3, NG - 2, NG - 1):
                        emit_T(gg)
                    for gg in (NG - 4, NG - 3, NG - 2, NG - 1):
                        emit_outproj(gg)
                if STAGE >= 3:
                    while q0_done < len(q0_tasks):
                        q0_batch(*q0_tasks[q0_done])
                        q0_done += 1

                if STAGE >= 4 and g0_pend[0]:
                    g0_pend[0] = False
                    emit_g0()
                    emit_T(0)
                    emit_outproj(0)

    return _finish(nc)


def _finish(nc):
    _split_sync_waits(nc)
    return nc


# ---------------------------------------------------------------- host glue
def shard_inputs(x, Wq, bq, Wk, bk, Wv, bv, Wo, bo):
    """Full inputs -> per-core in_maps. Core c: batch c//2, head-half c%2."""
    import ml_dtypes

    DM = Wq.shape[0]
    DO = Wq.shape[1] // 2
    HL = DO // 64
    DOV = HL * 65
    NJ = DO // 128
    in_maps = []
    cache = {}
    idn = np.eye(128, dtype=ml_dtypes.bfloat16)
    for core in range(N_CORES):
        b, g = core // 2, core % 2
        if g not in cache:
            sl = slice(g * DO, (g + 1) * DO)
            wvp = np.zeros((DM, DOV), np.float32)
            bvb_row = np.zeros((DOV,), np.float32)
            for h in range(HL):
                wvp[:, h * 65 : h * 65 + 64] = Wv[:, g * DO + h * 64 : g * DO + (h + 1) * 64]
                bvb_row[h * 65 : h * 65 + 64] = bv[g * DO + h * 64 : g * DO + (h + 1) * 64]
                bvb_row[h * 65 + 64] = 1.0
            lmask = np.zeros((128, 128), np.float32)
            rmask = np.zeros((128, 512), np.float32)
            for hh in range(2):
                for c in range(64):
                    if c < 32:
                        lmask[hh * 64 + c, 0:64] = 1.0
                        rmask[hh * 64 + c, 64:128] = -31.25
                        rmask[hh * 64 + c, 320:384] = -31.25
                    else:
                        lmask[hh * 64 + c, 64:128] = 1.0
                        rmask[hh * 64 + c, 128:192] = -31.25
                        rmask[hh * 64 + c, 384:448] = -31.25
            cache[g] = dict(
                lm=lmask.astype(ml_dtypes.bfloat16),
                rm=rmask.astype(ml_dtypes.bfloat16),
                wq=np.ascontiguousarray(Wq[:, sl]).astype(np.float16),
                wk=np.ascontiguousarray(Wk[:, sl]).astype(np.float16),
                wvp=wvp.astype(np.float16),
                wo=np.ascontiguousarray(Wo[sl, :]).astype(ml_dtypes.bfloat16),
                bq=np.ascontiguousarray(bq[sl].reshape(NJ, 128).T),
                bk=np.ascontiguousarray(bk[sl].reshape(NJ, 128).T),
                bvb=np.broadcast_to(bvb_row, (128, DOV)).copy(),
                idn=idn,
            )
        m = dict(cache[g])
        m["xt"] = np.ascontiguousarray(x[b].T).astype(np.float16)
        import hashlib
        _nonce = 1 + int(hashlib.sha256(open(__file__, "rb").read()).hexdigest(), 16) % 509
        m["nonce"] = np.zeros((1, _nonce), np.float32)
        in_maps.append(m)
    return in_maps


_NC_CACHE = {}


def kernel(x, Wq, bq, Wk, bk, Wv, bv, Wo, bo):
    global LAST_EXEC_NS
    x = np.asarray(x, dtype=np.float32)
    Wq, bq = np.asarray(Wq, np.float32), np.asarray(bq, np.float32)
    Wk, bk = np.asarray(Wk, np.float32), np.asarray(bk, np.float32)
    Wv, bv = np.asarray(Wv, np.float32), np.asarray(bv, np.float32)
    Wo, bo = np.asarray(Wo, np.float32), np.asarray(bo, np.float32)
    B, NT, DM = x.shape

    from concourse.bass_utils import run_bass_kernel_spmd

    key = (NT, DM)
    if key not in _NC_CACHE:
        _NC_CACHE[key] = build_kernel(NT=NT, DM=DM)
    nc = _NC_CACHE[key]

    in_maps = shard_inputs(x, Wq, bq, Wk, bk, Wv, bv, Wo, bo)
    trace = bool(int(os.environ.get("BSATTN_TRACE", "0")))
    res = run_bass_kernel_spmd(nc, in_maps, list(range(N_CORES)), trace=trace)
    LAST_EXEC_NS = res.exec_time_ns
    globals()["LAST_RESULT"] = res

    out = np.empty((B, NT, DM), np.float32)
    for b in range(B):
        out[b] = res.results[2 * b]["y"] + res.results[2 * b + 1]["y"] + bo
    return out



# revision 16
# speedup vs baseline: 1.0306x; 1.0306x over previous
"""TRN2 Bass kernel for block-sparse attention (nn_BlockSparseAttention).

kernel(**inputs) takes the FULL unsharded inputs (x [4,4096,1024], Wq/Wk/Wv/Wo
[1024,1024], bq/bk/bv/bo [1024]) and returns the full output [4,4096,1024].

Sharding: 8 cores = 4 batches x 2 head-halves (8 heads each). Each core
computes QKV projections, block-sparse attention, and a partial
out-projection [4096,1024]; the host sums the two half-partials plus bo.

v3 design (fused single pipeline):
  - projections and attention are interleaved over 8 token chunks of 512:
    the 512-col projection matmuls keep the PE HAM clock-gate at 8/8
    (2.4 GHz) and hide the Act exp / DVE division latency of the
    attention stages, which previously ran in a separate phase at K=4/8
  - S^T matmuls (64-row contraction) for the two head-halves are emitted
    interleaved so they run CONCURRENTLY in disjoint PE row-groups
    (tile_position auto-derived from base_partition) -- ~2x on S
  - no mask matmuls: every S matmul is its own accumulation group and the
    invalid 64x64 window corners are zeroed on the idle GpSimd engine
    after the exp
  - block-0 global attention is split into small (j, 4-piece) tasks spread
    across the whole pipeline; a few out-projections are deferred to keep
    the PE dense through the serial tail (g0 / edge group)
  - HAM pre-warm: dummy matmuls on the identity tile during the initial
    weight/x DMA wait
"""
import os

import numpy as np

import concourse.bass as bass
import concourse.tile as tile
from concourse import mybir

F32 = mybir.dt.float32
F16 = mybir.dt.float16
BF16 = mybir.dt.bfloat16
AF = mybir.ActivationFunctionType
SCALE = 1.0 / 8.0  # 1/sqrt(Dh=64)

N_CORES = 8
LAST_EXEC_NS = None


def _split_sync_waits(nc, cap=1):
    """This walrus build rejects >cap sync waits on one instruction; move
    excess waits onto same-engine no-ops placed just before (waits only
    become stricter in order, so this is semantics-preserving)."""
    for fn in nc.m.functions:
        for bb in fn.blocks:
            out = []
            for inst in bb.instructions:
                si = inst.sync_info
                waits = list(si.on_wait) if si and si.on_wait else []
                if len(waits) > cap:
                    extra, keep = waits[:-cap], waits[-cap:]
                    for i in range(0, len(extra), cap):
                        nop = mybir.InstNoOp(
                            name=nc.get_next_instruction_name(),
                            engine=inst.engine,
                            ins=[],
                            outs=[],
                            sync_info=mybir.SyncInfo(
                                on_wait=extra[i : i + cap], on_update=[]
                            ),
                        )
                        nc.register_instruction(nop)
                        out.append(nop)
                    si.on_wait = keep
                out.append(inst)
            bb.instructions[:] = out


def build_kernel(NT=4096, DM=1024, HL=8, DMO=1024):
    """One-core program; SPMD across 8 cores with different input slices."""
    DO = HL * 64          # local head dims (512)
    DOV = HL * 65         # v with interleaved ones columns (520)
    KC = DM // 128        # d_model chunks (8)
    NJ = DO // 128        # head pairs (4)
    STOK = 512
    NS = NT // STOK       # 8
    NG = NT // 128        # 32 token chunks / query groups
    NCH = NG + 1          # 33 shifted v chunks

    nc = bass.Bass()
    # cache-buster: some compile caches in this stack key on the HLO
    # interface only (not the embedded BIR), so a stale executable from a
    # previous kernel version can be wrongly reused. A source-hash-sized
    # dummy input makes every kernel edit change the HLO signature.
    import hashlib
    _nonce = 1 + int(hashlib.sha256(open(__file__, "rb").read()).hexdigest(), 16) % 509
    nonce_d = nc.dram_tensor("nonce", [1, _nonce], F32, kind="ExternalInput")
    xt_d = nc.dram_tensor("xt", [DM, NT], F16, kind="ExternalInput")
    wq_d = nc.dram_tensor("wq", [DM, DO], F16, kind="ExternalInput")
    wk_d = nc.dram_tensor("wk", [DM, DO], F16, kind="ExternalInput")
    wv_d = nc.dram_tensor("wvp", [DM, DOV], F16, kind="ExternalInput")
    wo_d = nc.dram_tensor("wo", [DO, DMO], BF16, kind="ExternalInput")
    bq_d = nc.dram_tensor("bq", [128, NJ], F32, kind="ExternalInput")
    bk_d = nc.dram_tensor("bk", [128, NJ], F32, kind="ExternalInput")
    bvb_d = nc.dram_tensor("bvb", [128, DOV], F32, kind="ExternalInput")
    idn_d = nc.dram_tensor("idn", [128, 128], BF16, kind="ExternalInput")
    y_d = nc.dram_tensor("y", [NT, DMO], F32, kind="ExternalOutput")

    with tile.TileContext(nc) as tc, nc.allow_low_precision(
        reason="attention operands intentionally bf16/fp16; matmul accum f32"
    ):
        from contextlib import ExitStack

        with ExitStack() as ctx:
            pers = ctx.enter_context(tc.tile_pool(name="pers", bufs=1))
            xp = ctx.enter_context(tc.tile_pool(name="xp", bufs=2))
            etp = ctx.enter_context(tc.tile_pool(name="etp", bufs=24))
            valp = ctx.enter_context(tc.tile_pool(name="valp", bufs=3))
            astp = ctx.enter_context(tc.tile_pool(name="astp", bufs=10))
            atp = ctx.enter_context(tc.tile_pool(name="atp", bufs=14))
            dip = ctx.enter_context(tc.tile_pool(name="dip", bufs=8))
            ysp = ctx.enter_context(tc.tile_pool(name="ysp", bufs=4))
            psp = ctx.enter_context(
                tc.tile_pool(name="psp", bufs=2, space="PSUM")
            )

            kts = [pers.tile([128, NT], BF16, tag=f"kt{j}", name=f"kt{j}")
                   for j in range(NJ)]
            qts = [pers.tile([128, NT], BF16, tag=f"qt{j}", name=f"qt{j}")
                   for j in range(NJ)]
            # shifted v: chunk m = tokens [128m-64, 128m+64); per-j cols 130
            vs = pers.tile([128, NCH * DOV], BF16, tag="vs")
            idn = pers.tile([128, 128], BF16, tag="idn")
            # block-0 PV accumulators (SBUF, accumulated task-wise)
            oq0 = [pers.tile([64, 130], F32, tag=f"oq{j}", name=f"oq{j}")
                   for j in range(NJ)]
            wqs = pers.tile([128, KC * DO], F16, tag="wqs")
            wks = pers.tile([128, KC * DO], F16, tag="wks")
            wvs = pers.tile([128, KC * DOV], F16, tag="wvs")
            bqs = pers.tile([128, NJ], F32, tag="bqs")
            bks = pers.tile([128, NJ], F32, tag="bks")
            bvbs = pers.tile([128, DOV], F32, tag="bvbs")
            wos = [pers.tile([128, DMO], BF16, tag=f"wo{j}", name=f"wo{j}")
                   for j in range(NJ)]

            # ---------------- prologue DMAs ----------------
            nc.sync.dma_start(idn[:], idn_d[:])
            xts_tiles = {}
            xts_tiles[0] = xp.tile([128, KC * STOK], F16, tag="xts",
                                   name="xts0")
            for c in range(KC):
                nc.sync.dma_start(
                    xts_tiles[0][:, c * STOK : (c + 1) * STOK],
                    xt_d[c * 128 : (c + 1) * 128, 0:STOK],
                )
            for c in range(KC):
                r = slice(c * 128, (c + 1) * 128)
                nc.sync.dma_start(wqs[:, c * DO : (c + 1) * DO], wq_d[r, :])
            nc.sync.dma_start(bqs[:], bq_d[:])
            for c in range(KC):
                r = slice(c * 128, (c + 1) * 128)
                nc.sync.dma_start(wks[:, c * DO : (c + 1) * DO], wk_d[r, :])
            nc.sync.dma_start(bks[:], bk_d[:])
            for c in range(KC):
                r = slice(c * 128, (c + 1) * 128)
                nc.sync.dma_start(wvs[:, c * DOV : (c + 1) * DOV], wv_d[r, :])
            nc.sync.dma_start(bvbs[:], bvb_d[:])
            for j in range(NJ):
                nc.sync.dma_start(wos[j][:], wo_d[j * 128 : (j + 1) * 128, :])
            # zero the phantom halves of the shifted-v layout so padded
            # full-row PV chains multiply by 0 instead of garbage
            nc.vector.memset(vs[0:64, 0:DOV], 0.0)
            nc.vector.memset(vs[64:128, NG * DOV : (NG + 1) * DOV], 0.0)

            # HAM pre-warm: ~3.4us of dummy matmuls during the DMA wait so
            # the first real chains run at K=8/8. Uses the first psY slot;
            # nothing reads it.
            NPW = int(os.environ.get("BSV3_PREWARM", "40"))
            if NPW:
                pw = psp.tile([128, 512], F32, tag="psY", name="prewarm",
                              bufs=1)
                for i in range(NPW):
                    nc.tensor.matmul(pw[:, 0:128], idn[:], idn[:],
                                     start=(i == 0), stop=(i == NPW - 1))

            # ---------------- helpers ----------------
            def vcol(m, j, hh):
                return m * DOV + j * 130 + hh * 65

            ets_store = {}  # pair g1 -> {(j, hh): (tile, qoff)}
            as_tiles = {}   # g -> [ast per j]
            at_tiles = {}   # g -> at tile

            def emit_S2j(g1, j):
                """S^T for group pair (g1, g1+1), head pair j, both halves
                interleaved so the two 64-row chains run concurrently in
                disjoint PE row-groups. Tile layout per (j, hh):
                [ca x q1 | cm x q1 | cm x q2 | cb x q2], 128 cols each.
                Every matmul is its own accumulation group (no column
                overlap); the 4 invalid 64x64 corners are zeroed by GpSimd
                after the exp."""
                g2 = g1 + 1
                ka = 128 * g1 - 64
                km = 128 * g1 + 64
                kb = 128 * g2 + 64
                ps = {}
                for hh in (0, 1):
                    ps[hh] = psp.tile([128, 512], F32, tag="psS",
                                      name=f"psS_{g1}_{j}_{hh}")
                if os.environ.get("BSV3_SEQS2"):
                    order = [(seg, hh) for hh in (0, 1) for seg in range(3)]
                else:
                    order = [(seg, hh) for seg in range(3) for hh in (0, 1)]
                segs = ((ka, 0, 128), (km, 0, 256), (kb, 128, 128))
                for seg, hh in order:
                    ko, qo, qn = segs[seg]
                    co = (0, 128, 384)[seg]
                    hr = slice(hh * 64, hh * 64 + 64)
                    nc.tensor.matmul(
                        ps[hh][:, co : co + qn],
                        kts[j][hr, ko : ko + 128],
                        qts[j][hr, 128 * g1 + qo : 128 * g1 + qo + qn],
                        start=True, stop=True,
                    )
                st = ets_store.setdefault(g1, {})
                MS = os.environ.get("BSV3_MEMSET", "gpsimd")
                for hh in (0, 1):
                    et = etp.tile([128, 512], BF16, tag="et",
                                  name=f"et_{g1}_{j}_{hh}")
                    nc.scalar.activation(et[:], ps[hh][:], AF.Exp,
                                         scale=SCALE)
                    eng = {"gpsimd": nc.gpsimd, "vector": nc.vector,
                           "none": None}[MS]
                    if eng is not None:
                        eng.memset(et[0:64, 64:128], 0.0)
                        eng.memset(et[64:128, 128:192], 0.0)
                        eng.memset(et[0:64, 320:384], 0.0)
                        eng.memset(et[64:128, 384:448], 0.0)
                    st[(j, hh)] = (et, 0)
                    st[(j, hh, "g2")] = (et, 256)

            def emit_S31j(j):
                """Edge group 31 (no right neighbor), head pair j. Layout
                per hh half-tile: [ca x q | cb x q], cb only 64 keys."""
                g = NG - 1
                ka = 128 * g - 64
                kb = 128 * g + 64
                ps = {}
                for hh in (0, 1):
                    ps[hh] = psp.tile([128, 512], F32, tag="psS",
                                      name=f"psS31_{j}_{hh}")
                # adjacent matmuls in disjoint row-groups run concurrently
                # and must drain to different PSUM banks
                for hh in (0, 1):
                    hr = slice(hh * 64, hh * 64 + 64)
                    nc.tensor.matmul(
                        ps[hh][:, 0:128],
                        kts[j][hr, ka : ka + 128],
                        qts[j][hr, 128 * g : 128 * g + 128],
                        start=True, stop=True,
                    )
                for hh in (0, 1):
                    hr = slice(hh * 64, hh * 64 + 64)
                    nc.tensor.matmul(
                        ps[hh][0:64, 128:256],
                        kts[j][hr, kb : kb + 64],
                        qts[j][hr, 128 * g : 128 * g + 128],
                        start=True, stop=True,
                    )
                st = ets_store.setdefault("e31", {})
                et = etp.tile([128, 512], BF16, tag="et", name=f"et31_{j}")
                for hh in (0, 1):
                    o = hh * 256
                    nc.scalar.activation(et[:, o : o + 128],
                                         ps[hh][:, 0:128],
                                         AF.Exp, scale=SCALE)
                    nc.scalar.activation(et[0:64, o + 128 : o + 256],
                                         ps[hh][0:64, 128:256],
                                         AF.Exp, scale=SCALE)
                    nc.gpsimd.memset(et[0:64, o + 64 : o + 128], 0.0)
                    nc.gpsimd.memset(et[64:128, o + 128 : o + 256], 0.0)
                    st[(j, hh)] = (et, hh * 256)
                return st

            def emit_PV(g, ets):
                """merged PV, batched reciprocal, division on DVE."""
                as_tiles[g] = []
                for j in range(NJ):
                    jj = j % 2
                    if jj == 0:
                        pv = psp.tile([128, 512], F32, tag="psPV",
                                      name=f"psPV_{g}_{j}")
                    c0 = jj * 130
                    for hh in (0, 1):
                        et, off = ets[(j, hh)]
                        co = c0 + hh * 65
                        nc.tensor.matmul(
                            pv[:, co : co + 65],
                            et[:, off : off + 128],
                            vs[:, vcol(g, j, hh) : vcol(g, j, hh) + 65],
                            start=True, stop=False,
                        )
                        nc.tensor.matmul(
                            pv[:, co : co + 65],
                            et[:, off + 128 : off + 256],
                            vs[:, vcol(g + 1, j, hh) : vcol(g + 1, j, hh) + 65],
                            start=False, stop=True,
                        )
                    if jj == 1:
                        dinv = dip.tile([128, 4], F32, tag="dinv",
                                        name=f"dinv_{g}_{j}")
                        nc.vector.reciprocal(
                            dinv[:],
                            pv[:, 0:260].rearrange(
                                "p (h c) -> p h c", c=65
                            )[:, :, 64:65],
                        )
                        for j2 in (j - 1, j):
                            ast = astp.tile([128, 128], BF16, tag="ast",
                                            name=f"ast_{g}_{j2}")
                            cb = (j2 % 2) * 130
                            for hh in (0, 1):
                                nc.vector.tensor_scalar_mul(
                                    ast[:, hh * 64 : hh * 64 + 64],
                                    pv[:, cb + hh * 65 : cb + hh * 65 + 64],
                                    dinv[:, (j2 % 2) * 2 + hh : (j2 % 2) * 2 + hh + 1],
                                )
                            as_tiles[g].append(ast)

            def emit_T(g):
                """PE-transpose astage -> one A^T tile [128, 4*128]."""
                ptw = 512 if os.environ.get("BSV3_PST512") else 1024
                pt = psp.tile([128, ptw], BF16, tag="psT", name=f"psT_{g}",
                              bufs=1)
                for j in range(NJ):
                    nc.tensor.transpose(
                        pt[:, j * 128 : (j + 1) * 128], as_tiles[g][j], idn
                    )
                att = atp.tile([128, 512], BF16, tag="at", name=f"at_{g}")
                nc.vector.tensor_copy(att[:], pt[:, 0:512])
                at_tiles[g] = att
                del as_tiles[g]

            def emit_outproj_half(g, n):
                py = psp.tile([128, 512], F32, tag="psY", name=f"psY_{g}_{n}",
                              bufs=1)
                for j in range(NJ):
                    nc.tensor.matmul(
                        py[:],
                        at_tiles[g][:, j * 128 : (j + 1) * 128],
                        wos[j][:, n * 512 : n * 512 + 512],
                        start=(j == 0), stop=(j == NJ - 1),
                    )
                ysb = ysp.tile([128, 512], F32, tag="ysb", name=f"ysb_{g}_{n}")
                if (g + n) % 2 == 0:
                    nc.scalar.copy(ysb[:], py[:])
                else:
                    nc.vector.tensor_copy(ysb[:], py[:])
                nc.sync.dma_start(
                    y_d[g * 128 : (g + 1) * 128, n * 512 : n * 512 + 512],
                    ysb[:],
                )
                if n == 1:
                    del at_tiles[g]

            # ----- block-0 global attention tasks -----
            def q0_edge0(j):
                """piece 0 (keys 0:64 live on partitions 64:128 of chunk 0);
                initializes oq0[j] via tensor_copy."""
                ps = {}
                for hh in (0, 1):
                    ps[hh] = psp.tile([128, 512], F32, tag="psS",
                                      name=f"q0e0_{j}_{hh}")
                    hr = slice(hh * 64, hh * 64 + 64)
                    nc.tensor.matmul(
                        ps[hh][64:128, 0:64],
                        kts[j][hr, 0:64], qts[j][hr, 0:64],
                        start=True, stop=True,
                    )
                eq = etp.tile([128, 512], BF16, tag="et", name=f"eq0_{j}")
                for hh in (0, 1):
                    nc.scalar.activation(eq[64:128, hh * 64 : hh * 64 + 64],
                                         ps[hh][64:128, 0:64],
                                         AF.Exp, scale=SCALE)
                pv = psp.tile([128, 512], F32, tag="psPV", name=f"pvq0_{j}")
                for hh in (0, 1):
                    nc.tensor.matmul(
                        pv[0:64, hh * 65 : hh * 65 + 65],
                        eq[64:128, hh * 64 : hh * 64 + 64],
                        vs[64:128, vcol(0, j, hh) : vcol(0, j, hh) + 65],
                        start=True, stop=True,
                    )
                nc.vector.tensor_copy(oq0[j][:], pv[0:64, 0:130])

            def q0_edge32(j):
                """piece 32 (keys 4032:4096 on partitions 0:64 of chunk 32)."""
                ps = {}
                for hh in (0, 1):
                    ps[hh] = psp.tile([128, 512], F32, tag="psS",
                                      name=f"q0e32_{j}_{hh}")
                    hr = slice(hh * 64, hh * 64 + 64)
                    nc.tensor.matmul(
                        ps[hh][0:64, 0:64],
                        kts[j][hr, NT - 64 : NT], qts[j][hr, 0:64],
                        start=True, stop=True,
                    )
                eq = etp.tile([128, 512], BF16, tag="et", name=f"eq32_{j}")
                for hh in (0, 1):
                    nc.scalar.activation(eq[0:64, hh * 64 : hh * 64 + 64],
                                         ps[hh][0:64, 0:64],
                                         AF.Exp, scale=SCALE)
                pv = psp.tile([128, 512], F32, tag="psPV", name=f"pvq32_{j}")
                for hh in (0, 1):
                    nc.tensor.matmul(
                        pv[0:64, hh * 65 : hh * 65 + 65],
                        eq[0:64, hh * 64 : hh * 64 + 64],
                        vs[0:64, vcol(NG, j, hh) : vcol(NG, j, hh) + 65],
                        start=True, stop=True,
                    )
                nc.vector.tensor_add(oq0[j][:], oq0[j][:], pv[0:64, 0:130])

            def q0_range(j, r):
                """block-0 vs pieces 4r+1..4r+4 (clipped to 31) for head
                pair j. S matmuls packed across hh row-groups; one exp; PV
                chains per hh accumulated into oq0[j]."""
                pieces = [m for m in range(4 * r + 1, 4 * r + 5) if m <= NG - 1]
                ps = {}
                for hh in (0, 1):
                    ps[hh] = psp.tile([128, 512], F32, tag="psS",
                                      name=f"q0r_{j}_{r}_{hh}")
                for c, m in enumerate(pieces):
                    for hh in (0, 1):
                        hr = slice(hh * 64, hh * 64 + 64)
                        nc.tensor.matmul(
                            ps[hh][:, c * 64 : c * 64 + 64],
                            kts[j][hr, 128 * m - 64 : 128 * m + 64],
                            qts[j][hr, 0:64],
                            start=True, stop=True,
                        )
                eq = etp.tile([128, 512], BF16, tag="et", name=f"eqr_{j}_{r}")
                nw = len(pieces) * 64
                for hh in (0, 1):
                    nc.scalar.activation(
                        eq[:, hh * 256 : hh * 256 + nw],
                        ps[hh][:, 0:nw],
                        AF.Exp, scale=SCALE,
                    )
                pv = psp.tile([128, 512], F32, tag="psPV", name=f"pvr_{j}_{r}")
                for hh in (0, 1):
                    for c, m in enumerate(pieces):
                        nc.tensor.matmul(
                            pv[0:64, hh * 65 : hh * 65 + 65],
                            eq[:, hh * 256 + c * 64 : hh * 256 + c * 64 + 64],
                            vs[:, vcol(m, j, hh) : vcol(m, j, hh) + 65],
                            start=(c == 0), stop=(c == len(pieces) - 1),
                        )
                nc.vector.tensor_add(oq0[j][:], oq0[j][:], pv[0:64, 0:130])

            def emit_g0():
                """group 0: block 0 (rows 0:64, from the accumulated global
                pass) + block 1 (rows 64:128, local window {0,1,2})."""
                as_tiles[0] = []
                ega, egb = {}, {}
                for hh in (0, 1):
                    hr = slice(hh * 64, hh * 64 + 64)
                    psA = psp.tile([128, 512], F32, tag="psS",
                                   name=f"g0a_{hh}")
                    for j in range(NJ):
                        nc.tensor.matmul(
                            psA[64:128, j * 64 : j * 64 + 64],
                            kts[j][hr, 0:64], qts[j][hr, 64:128],
                            start=True, stop=True,
                        )
                        nc.tensor.matmul(
                            psA[:, 256 + j * 64 : 256 + j * 64 + 64],
                            kts[j][hr, 64:192], qts[j][hr, 64:128],
                            start=True, stop=True,
                        )
                    ea = etp.tile([128, 512], BF16, tag="et", name=f"ga_{hh}")
                    nc.scalar.activation(
                        ea[64:128, 0:256], psA[64:128, 0:256],
                        AF.Exp, scale=SCALE,
                    )
                    nc.scalar.activation(ea[:, 256:512], psA[:, 256:512],
                                         AF.Exp, scale=SCALE)
                    nc.gpsimd.memset(ea[0:64, 0:256], 0.0)
                    ega[hh], egb[hh] = ea, ea
                for j in range(NJ):
                    jj = j % 2
                    if jj == 0:
                        pv0 = psp.tile([128, 512], F32, tag="psPV",
                                       name=f"pv0_{j}")
                    c0 = jj * 130
                    for hh in (0, 1):
                        nc.tensor.matmul(
                            pv0[64:128, c0 + hh * 65 : c0 + hh * 65 + 65],
                            ega[hh][:, j * 64 : j * 64 + 64],
                            vs[:, vcol(0, j, hh) : vcol(0, j, hh) + 65],
                            start=True, stop=False,
                        )
                        nc.tensor.matmul(
                            pv0[64:128, c0 + hh * 65 : c0 + hh * 65 + 65],
                            egb[hh][:, 256 + j * 64 : 256 + j * 64 + 64],
                            vs[:, vcol(1, j, hh) : vcol(1, j, hh) + 65],
                            start=False, stop=True,
                        )
                    dinv = dip.tile([128, 4], F32, tag="dinv",
                                    name=f"dinv0_{j}")
                    for hh in (0, 1):
                        nc.vector.reciprocal(
                            dinv[64:128, hh : hh + 1],
                            pv0[64:128, c0 + hh * 65 + 64 : c0 + hh * 65 + 65],
                        )
                        nc.vector.reciprocal(
                            dinv[0:64, hh : hh + 1],
                            oq0[j][:, hh * 65 + 64 : hh * 65 + 65],
                        )
                    ast = astp.tile([128, 128], BF16, tag="ast",
                                    name=f"ast0_{j}")
                    for hh in (0, 1):
                        nc.vector.tensor_scalar_mul(
                            ast[64:128, hh * 64 : hh * 64 + 64],
                            pv0[64:128, c0 + hh * 65 : c0 + hh * 65 + 64],
                            dinv[64:128, hh : hh + 1],
                        )
                        nc.vector.tensor_scalar_mul(
                            ast[0:64, hh * 64 : hh * 64 + 64],
                            oq0[j][:, hh * 65 : hh * 65 + 64],
                            dinv[0:64, hh : hh + 1],
                        )
                    as_tiles[0].append(ast)

            # ---------------- task scheduler ----------------
            # slot L = 16*s + i; tasks emit after the slot's proj chain when
            # ready_L <= L. Insertion order respects intra-group deps.
            tasks = []
            LEVEL = int(os.environ.get("BSV3_LEVEL", "99"))

            def add(ready, cost, fn, lvl=0):
                if lvl > LEVEL:
                    return
                tasks.append([ready, cost, fn, False])

            def pump(L, budget):
                spent = 0.0
                for t in tasks:
                    if t[3] or t[0] > L:
                        continue
                    t[2]()
                    t[3] = True
                    spent += t[1]
                    if spent >= budget:
                        break

            def vslot(T):
                """slot index after which vs chunk T's scatter DMA is
                emitted (+1 slot of lag for the DMA to land)."""
                s, t = T // 4, T % 4
                return 16 * s + 10 + 2 * t

            DEFER = {21, 22, 23, 24}

            def pv1_ets(g1):
                return {k: v for k, v in ets_store[g1].items()
                        if len(k) == 2}

            def pv2_ets(g1):
                return {(j, hh): ets_store[g1][(j, hh, "g2")]
                        for j in range(NJ) for hh in (0, 1)}

            # pair tasks
            for g1 in range(1, NG - 1, 2):
                g2 = g1 + 1
                sready = 16 * ((g2 + 1) // 4) + 8
                for j in range(NJ):
                    add(sready + j // 2, 0.45,
                        (lambda g1=g1, j=j: emit_S2j(g1, j)), lvl=1)
                pvr1 = max(sready + 3, vslot(g1 + 1) + 1)
                pvr2 = max(pvr1 + 1, vslot(g2 + 1) + 1)
                add(pvr1, 0.6,
                    (lambda g1=g1: emit_PV(g1, pv1_ets(g1))), lvl=2)
                add(pvr2, 0.6,
                    (lambda g1=g1, g2=g2: emit_PV(g2, pv2_ets(g1))), lvl=2)
                add(pvr2 + 1, 0.3, (lambda g1=g1: emit_T(g1)), lvl=3)
                add(pvr2 + 1, 0.3, (lambda g2=g2: emit_T(g2)), lvl=3)
                for gg, base in ((g1, pvr2 + 3), (g2, pvr2 + 4)):
                    if gg in DEFER:
                        base = 129 + 2 * (gg - 21)
                    add(base, 0.5,
                        (lambda gg=gg: emit_outproj_half(gg, 0)), lvl=4)
                    add(base + 1, 0.5,
                        (lambda gg=gg: emit_outproj_half(gg, 1)), lvl=4)

            # edge group 31
            S31LVL = 9 if os.environ.get("BSV3_NOS31") else 1
            for j in range(NJ):
                add(16 * 7 + 8 + j // 2, 0.35, (lambda j=j: emit_S31j(j)),
                    lvl=S31LVL)
            add(vslot(NG - 1) + 2, 0.6, (lambda: emit_PV(
                NG - 1, {k: v for k, v in ets_store["e31"].items()})), lvl=2)
            add(vslot(NG - 1) + 3, 0.3, (lambda: emit_T(NG - 1)), lvl=3)
            add(vslot(NG - 1) + 4, 0.5,
                (lambda: emit_outproj_half(NG - 1, 0)), lvl=4)
            add(vslot(NG - 1) + 5, 0.5,
                (lambda: emit_outproj_half(NG - 1, 1)), lvl=4)

            # block-0 tasks
            for j in range(NJ):
                add(11 + j // 2, 0.2, (lambda j=j: q0_edge0(j)), lvl=5)
            for r in range(8):
                for j in range(NJ):
                    rd = max(16 * min(r + 1, NS - 1) + 8,
                             vslot(min(4 * r + 4, NG - 1)) + 1)
                    add(rd + j, 0.45, (lambda j=j, r=r: q0_range(j, r)),
                        lvl=5)
            for j in range(NJ):
                add(vslot(NG - 1) + 2 + j, 0.2, (lambda j=j: q0_edge32(j)),
                    lvl=5)
            add(137, 2.0, emit_g0, lvl=6)
            add(138, 0.3, (lambda: emit_T(0)), lvl=6)
            add(139, 0.5, (lambda: emit_outproj_half(0, 0)), lvl=6)
            add(140, 0.5, (lambda: emit_outproj_half(0, 1)), lvl=6)

            # ---------------- the fused pipeline ----------------
            for s in range(NS):
                ts = slice(s * STOK, (s + 1) * STOK)
                L0 = 16 * s
                if s + 1 < NS:
                    xts_tiles[s + 1] = xp.tile([128, KC * STOK], F16,
                                               tag="xts", name=f"xts{s + 1}")
                    for c in range(KC):
                        nc.sync.dma_start(
                            xts_tiles[s + 1][:, c * STOK : (c + 1) * STOK],
                            xt_d[c * 128 : (c + 1) * 128,
                                 (s + 1) * STOK : (s + 2) * STOK],
                        )
                xts = xts_tiles.pop(s)
                for qi, (wsb, bsb, dsts) in enumerate(
                    ((wqs, bqs, qts), (wks, bks, kts))
                ):
                    for j in range(NJ):
                        ps = psp.tile([128, 512], F32, tag="pp",
                                      name=f"pp_{s}_{qi}_{j}")
                        for c in range(KC):
                            nc.tensor.matmul(
                                ps[:],
                                wsb[:, c * DO + j * 128 : c * DO + (j + 1) * 128],
                                xts[:, c * STOK : (c + 1) * STOK],
                                start=(c == 0), stop=(c == KC - 1),
                            )
                        nc.scalar.activation(
                            dsts[j][:, ts], ps[:], AF.Identity,
                            bias=bsb[:, j : j + 1],
                        )
                        pump(L0 + 4 * qi + j, 1.1)
                for t in range(STOK // 128):
                    T = 4 * s + t
                    val = valp.tile([128, DOV], BF16, tag="val",
                                    name=f"val_{T}")
                    for oi, (o, wd) in enumerate(((0, 260), (260, 260))):
                        psv = psp.tile([128, 512], F32, tag="pp",
                                       name=f"ppv_{T}_{oi}")
                        for c in range(KC):
                            nc.tensor.matmul(
                                psv[:, 0:wd],
                                xts[:, c * STOK + t * 128 : c * STOK + (t + 1) * 128],
                                wvs[:, c * DOV + o : c * DOV + o + wd],
                                start=(c == 0), stop=(c == KC - 1),
                            )
                        nc.vector.tensor_add(
                            val[:, o : o + wd], bvbs[:, o : o + wd],
                            psv[:, 0:wd]
                        )
                        pump(L0 + 8 + 2 * t + oi, 0.9)
                    # scatter into shifted-chunk layout
                    nc.sync.dma_start(
                        vs[64:128, T * DOV : (T + 1) * DOV], val[0:64, :]
                    )
                    nc.sync.dma_start(
                        vs[0:64, (T + 1) * DOV : (T + 2) * DOV], val[64:128, :]
                    )
            # ---------------- tail ----------------
            L = 16 * NS
            while any(not t[3] for t in tasks):
                pump(L, 1.3)
                L += 1
                assert L < 400, "scheduler deadlock"

    return _finish(nc)


def _finish(nc):
    _split_sync_waits(nc)
    return nc


# ---------------------------------------------------------------- host glue
def shard_inputs(x, Wq, bq, Wk, bk, Wv, bv, Wo, bo):
    """Full inputs -> per-core in_maps. Core c: batch c//2, head-half c%2."""
    import ml_dtypes

    DM = Wq.shape[0]
    DO = Wq.shape[1] // 2
    HL = DO // 64
    DOV = HL * 65
    NJ = DO // 128
    in_maps = []
    cache = {}
    idn = np.eye(128, dtype=ml_dtypes.bfloat16)
    for core in range(N_CORES):
        b, g = core // 2, core % 2
        if g not in cache:
            sl = slice(g * DO, (g + 1) * DO)
            wvp = np.zeros((DM, DOV), np.float32)
            bvb_row = np.zeros((DOV,), np.float32)
            for h in range(HL):
                wvp[:, h * 65 : h * 65 + 64] = Wv[:, g * DO + h * 64 : g * DO + (h + 1) * 64]
                bvb_row[h * 65 : h * 65 + 64] = bv[g * DO + h * 64 : g * DO + (h + 1) * 64]
                bvb_row[h * 65 + 64] = 1.0
            cache[g] = dict(
                wq=np.ascontiguousarray(Wq[:, sl]).astype(np.float16),
                wk=np.ascontiguousarray(Wk[:, sl]).astype(np.float16),
                wvp=wvp.astype(np.float16),
                wo=np.ascontiguousarray(Wo[sl, :]).astype(ml_dtypes.bfloat16),
                bq=np.ascontiguousarray(bq[sl].reshape(NJ, 128).T),
                bk=np.ascontiguousarray(bk[sl].reshape(NJ, 128).T),
                bvb=np.broadcast_to(bvb_row, (128, DOV)).copy(),
                idn=idn,
            )
        m = dict(cache[g])
        m["xt"] = np.ascontiguousarray(x[b].T).astype(np.float16)
        import hashlib
        _nonce = 1 + int(hashlib.sha256(open(__file__, "rb").read()).hexdigest(), 16) % 509
        m["nonce"] = np.zeros((1, _nonce), np.float32)
        in_maps.append(m)
    return in_maps


_NC_CACHE = {}


def kernel(x, Wq, bq, Wk, bk, Wv, bv, Wo, bo):
    global LAST_EXEC_NS
    x = np.asarray(x, dtype=np.float32)
    Wq, bq = np.asarray(Wq, np.float32), np.asarray(bq, np.float32)
    Wk, bk = np.asarray(Wk, np.float32), np.asarray(bk, np.float32)
    Wv, bv = np.asarray(Wv, np.float32), np.asarray(bv, np.float32)
    Wo, bo = np.asarray(Wo, np.float32), np.asarray(bo, np.float32)
    B, NT, DM = x.shape

    from concourse.bass_utils import run_bass_kernel_spmd

    key = (NT, DM)
    if key not in _NC_CACHE:
        _NC_CACHE[key] = build_kernel(NT=NT, DM=DM)
    nc = _NC_CACHE[key]

    in_maps = shard_inputs(x, Wq, bq, Wk, bk, Wv, bv, Wo, bo)
    trace = bool(int(os.environ.get("BSATTN_TRACE", "0")))
    res = run_bass_kernel_spmd(nc, in_maps, list(range(N_CORES)), trace=trace)
    LAST_EXEC_NS = res.exec_time_ns
    globals()["LAST_RESULT"] = res

    out = np.empty((B, NT, DM), np.float32)
    for b in range(B):
        out[b] = res.results[2 * b]["y"] + res.results[2 * b + 1]["y"] + bo
    return out
